# revision 15
# baseline (speedup 1.0000x reference)
"""Multi-head attention kernel for Trainium2 (Bass/Tile), 8-core data parallel.

Problem: B=32, N=1024, D=512, H=8 (per-head dim = D = 512).
  kh = k @ Wk[h].T + bk ; vh = v @ Wv[h].T + bv ; qh = q @ Wq[h].T + bq
  S = qh @ kh.T / sqrt(D); P = softmax(S); out_h = P @ vh
  rep = concat_interleaved(out_h) @ Wo.T + bo

Sharding: batch data-parallel, 4 batches per core. All math per (b, h) is
done in "transposed" (feature-on-partition) orientation so no on-chip
transposes are needed:
  qhT[e,i]  = matmul(lhsT=WqT, rhs=qT)        (+ bq during PSUM eviction)
  khT[e,j]  = matmul(lhsT=WkT, rhs=kT)        (bk dropped: softmax-invariant)
  vh[j,d]   = matmul(lhsT=vT,  rhs=WvT)       (bv folded into bo on host)
  ST[j,i]   = matmul(lhsT=khT, rhs=qhT)
  E[j,i]    = exp(ST/sqrt(D))                  (no max-subtract: scores ~N(0,1))
  denom     = onesT @ E   (all-ones lhsT -> every row = column sum)
  outT[d,i] = matmul(lhsT=vh, rhs=E) * (1/denom)  (rescale on eviction)
  repT[eo,i]+= matmul(lhsT=WoT_h, rhs=outT)   (accumulate heads in SBUF)
  out = repT + bo_eff                          (host folds bv through Wo)

Matmul operands use float32r (full PE rate at free-dim>=256, ~1.5e-4 rel err).
"""
import math
from contextlib import ExitStack

import numpy as np

import concourse.bacc as bacc
import concourse.mybir as mybir
import concourse.tile as tile
from concourse.bass_utils import run_bass_kernel_spmd

dt = mybir.dt
P = 128

B, N, D, H = 32, 1024, 512, 8
NCORES = 8
BLOC = B // NCORES

FD = 512           # matmul free-dim / PSUM bank width (f32)
SCALE = 1.0 / math.sqrt(D)


class _Ctx:
    pass


def build_core_program(bloc=BLOC, n=N, d=D, h_cnt=H, reps=1, pe_only=False, no_dma=False, ps_s_bufs=3, ps_pv_bufs=4):
    """Bass program for one core: bloc batches, full heads."""
    c = _Ctx()
    c.DC = d // P        # d-partition chunks (4)
    c.EC = d // P        # output-feature chunks (4)
    c.IC = n // FD       # query free-dim chunks (2)
    c.JC8 = n // P       # key partition chunks (8)
    c.n, c.d, c.h_cnt = n, d, h_cnt
    c.pe_only = pe_only
    c.no_dma = no_dma

    nc = bacc.Bacc("TRN2", target_bir_lowering=False, debug=False)
    c.nc = nc

    f32, f32r = dt.float32, dt.float32r
    c.f32, c.f32r = f32, f32r
    c.qT = nc.dram_tensor("qT", [bloc, d, n], f32r, kind="ExternalInput")
    c.kT = nc.dram_tensor("kT", [bloc, d, n], f32r, kind="ExternalInput")
    c.vT = nc.dram_tensor("vT", [bloc, d, n], f32r, kind="ExternalInput")
    c.WqT = nc.dram_tensor("WqT", [h_cnt, d, d], f32r, kind="ExternalInput")
    c.WkT = nc.dram_tensor("WkT", [h_cnt, d, d], f32r, kind="ExternalInput")
    c.WvT = nc.dram_tensor("WvT", [h_cnt, d, d], f32r, kind="ExternalInput")
    c.WoT = nc.dram_tensor("WoT", [h_cnt, d, d], f32r, kind="ExternalInput")
    c.bq_d = nc.dram_tensor("bq_d", [P, h_cnt * c.EC], f32, kind="ExternalInput")
    c.bo_d = nc.dram_tensor("bo_d", [P, c.EC], f32, kind="ExternalInput")
    c.ones_d = nc.dram_tensor("ones_d", [P, P], f32r, kind="ExternalInput")
    c.outT = nc.dram_tensor("outT", [bloc, d, n], f32, kind="ExternalOutput")

    c.AF = mybir.ActivationFunctionType

    with tile.TileContext(nc) as tc, ExitStack() as es:
        ep = es.enter_context
        c.const = ep(tc.tile_pool(name="const", bufs=1))
        c.acts = ep(tc.tile_pool(name="acts", bufs=1))
        c.wqp = ep(tc.tile_pool(name="wq", bufs=2))
        c.wkp = ep(tc.tile_pool(name="wk", bufs=2))
        c.wvp = ep(tc.tile_pool(name="wv", bufs=2))
        c.wop = ep(tc.tile_pool(name="wo", bufs=2))
        c.projp = ep(tc.tile_pool(name="proj", bufs=1))
        c.esbp = ep(tc.tile_pool(name="esb", bufs=3))
        c.outnp = ep(tc.tile_pool(name="outn", bufs=1))
        c.recipp = ep(tc.tile_pool(name="recip", bufs=1))
        c.repp = ep(tc.tile_pool(name="rep", bufs=1))
        c.ps_s = ep(tc.tile_pool(name="ps_s", bufs=ps_s_bufs, space="PSUM"))
        c.ps_pv = ep(tc.tile_pool(name="ps_pv", bufs=ps_pv_bufs, space="PSUM"))
        c.ps_d = ep(tc.tile_pool(name="ps_d", bufs=1, space="PSUM"))

        c.ones = c.const.tile([P, P], f32r, name="ones")
        nc.sync.dma_start(c.ones[:], c.ones_d[:])
        c.bq_sb = c.const.tile([P, h_cnt * c.EC], f32, name="bq_sb")
        nc.sync.dma_start(c.bq_sb[:], c.bq_d[:])
        c.bo_sb = c.const.tile([P, c.EC], f32, name="bo_sb")
        nc.sync.dma_start(c.bo_sb[:], c.bo_d[:])
        if pe_only:
            c.d_qhT = c.const.tile([P, c.EC, n], f32r, name="d_qhT")
            c.d_khT = c.const.tile([P, c.EC, n], f32r, name="d_khT")
            c.d_vh = c.const.tile([P, c.JC8, FD], f32r, name="d_vh")
            c.d_e = c.const.tile([P, FD], f32r, name="d_e")
            c.d_outn = c.const.tile([P, c.DC, FD], f32r, name="d_outn")
            if no_dma:
                c.d_w = c.const.tile([P, c.DC, d], f32r, name="d_w")
                nc.sync.dma_start(c.d_w[:], c.WqT[0].rearrange("(c p) e -> p c e", p=P))
            nc.sync.dma_start(c.d_qhT[:], c.qT[0].rearrange("(c p) n -> p c n", p=P))
            nc.sync.dma_start(c.d_khT[:], c.kT[0].rearrange("(c p) n -> p c n", p=P))
            for j in range(c.JC8):
                nc.sync.dma_start(c.d_vh[:, j, :], c.vT[0, 0:P, 0:FD])
            nc.sync.dma_start(c.d_e[:], c.qT[0, 0:P, 0:FD])
            for dcx in range(c.DC):
                nc.sync.dma_start(c.d_outn[:, dcx, :], c.qT[0, 0:P, 0:FD])

        for rep in range(reps):
            for b in range(bloc):
                _emit_batch(c, b)

    nc.compile()
    return nc


def _emit_batch(c, b):
    nc = c.nc
    qt = c.acts.tile([P, c.DC, c.n], c.f32r, name="qt")
    kt = c.acts.tile([P, c.DC, c.n], c.f32r, name="kt")
    vt = c.acts.tile([P, c.DC, c.n], c.f32r, name="vt")
    # h=0 weights are issued first: the HWDGE queue is serial, and the
    # first projection group needs wq before anything else. Activation
    # chunks follow so proj(q, ec=0) can start after wq + one 512KB chunk.
    w0 = _issue_weight_dmas(c, 0)
    for dcx in range(c.DC):
        nc.sync.dma_start(qt[:, dcx, :], c.qT[b, dcx * P:(dcx + 1) * P, :])
        nc.sync.dma_start(kt[:, dcx, :], c.kT[b, dcx * P:(dcx + 1) * P, :])
        nc.sync.dma_start(vt[:, dcx, :], c.vT[b, dcx * P:(dcx + 1) * P, :])

    repT = c.repp.tile([P, c.EC, c.n], c.f32, name="repT")

    for h in range(c.h_cnt):
        _emit_head(c, h, qt, kt, vt, repT, w0 if h == 0 else None)

    for ec in range(c.EC):
        nc.vector.tensor_scalar_add(
            repT[:, ec, :], repT[:, ec, :], c.bo_sb[:, ec:ec + 1])
    nc.sync.dma_start(
        c.outT[b].rearrange("(c p) n -> p c n", p=P), repT[:])


def _issue_weight_dmas(c, h):
    nc = c.nc
    if c.no_dma:
        return (c.d_w,) * 4
    wq = c.wqp.tile([P, c.DC, c.d], c.f32r, name="wq")
    wk = c.wkp.tile([P, c.DC, c.d], c.f32r, name="wk")
    wv = c.wvp.tile([P, c.DC, c.d], c.f32r, name="wv")
    wo = c.wop.tile([P, c.DC, c.d], c.f32r, name="wo")
    nc.sync.dma_start(wq[:], c.WqT[h].rearrange("(c p) e -> p c e", p=P))
    nc.sync.dma_start(wk[:], c.WkT[h].rearrange("(c p) e -> p c e", p=P))
    nc.sync.dma_start(wv[:], c.WvT[h].rearrange("(c p) e -> p c e", p=P))
    nc.sync.dma_start(wo[:], c.WoT[h].rearrange("(c p) e -> p c e", p=P))
    return wq, wk, wv, wo


def _emit_head(c, h, qt, kt, vt, repT, w0=None):
    nc = c.nc
    DC, EC, IC, JC8 = c.DC, c.EC, c.IC, c.JC8

    wq, wk, wv, wo = w0 if w0 is not None else _issue_weight_dmas(c, h)

    # ---- projections ----
    if c.pe_only:
        qhT, khT, vh = c.d_qhT, c.d_khT, c.d_vh
    else:
        qhT = c.projp.tile([P, EC, c.n], c.f32r, name="qhT")
        khT = c.projp.tile([P, EC, c.n], c.f32r, name="khT")
        vh = c.projp.tile([P, JC8, FD], c.f32r, name="vh")

    for ec in range(EC):
        for ic in range(IC):
            pq = c.ps_s.tile([P, FD], c.f32, name="ps_s")
            for dc in range(DC):
                nc.tensor.matmul(
                    pq[:], wq[:, dc, ec * P:(ec + 1) * P],
                    qt[:, dc, ic * FD:(ic + 1) * FD],
                    start=(dc == 0), stop=(dc == DC - 1))
            if not c.pe_only:
                nc.scalar.activation(
                    qhT[:, ec, ic * FD:(ic + 1) * FD], pq[:], c.AF.Identity,
                    bias=c.bq_sb[:, h * EC + ec:h * EC + ec + 1])
    for ec in range(EC):
        for jc in range(IC):
            pk = c.ps_s.tile([P, FD], c.f32, name="ps_s")
            for dc in range(DC):
                nc.tensor.matmul(
                    pk[:], wk[:, dc, ec * P:(ec + 1) * P],
                    kt[:, dc, jc * FD:(jc + 1) * FD],
                    start=(dc == 0), stop=(dc == DC - 1))
            if not c.pe_only:
                nc.vector.tensor_copy(khT[:, ec, jc * FD:(jc + 1) * FD], pk[:])
    for jc8 in range(JC8):
        pv = c.ps_s.tile([P, FD], c.f32, name="ps_s")
        for dc in range(DC):
            nc.tensor.matmul(
                pv[:], vt[:, dc, jc8 * P:(jc8 + 1) * P], wv[:, dc, :],
                start=(dc == 0), stop=(dc == DC - 1))
        if not c.pe_only:
            nc.vector.tensor_copy(vh[:, jc8, :], pv[:])

    # ---- attention + output projection, per query chunk ----
    if c.pe_only:
        qhT, khT, vh = c.d_qhT, c.d_khT, c.d_vh
    for ic in range(IC):
        _emit_attention_chunk(c, h, ic, qhT, khT, vh, wo, repT)


def _emit_attention_chunk(c, h, ic, qhT, khT, vh, wo, repT):
    nc = c.nc
    DC, EC, JC8 = c.DC, c.EC, c.JC8
    i_sl = slice(ic * FD, (ic + 1) * FD)

    pv_ps = [c.ps_pv.tile([P, FD], c.f32, name="ps_pv") for _ in range(DC)]
    den_ps = c.ps_d.tile([P, FD], c.f32, name="ps_d")
    e_tiles = [None] * JC8

    # software-pipelined: S(j+1) issues on PE while exp(j) runs on ACT,
    # then denom/PV(j) consume.
    def issue_s(jc8):
        st = c.ps_s.tile([P, FD], c.f32, name="ps_s")
        for ec in range(EC):
            nc.tensor.matmul(
                st[:], khT[:, ec, jc8 * P:(jc8 + 1) * P], qhT[:, ec, i_sl],
                start=(ec == 0), stop=(ec == EC - 1))
        if c.pe_only:
            e_tiles[jc8] = c.d_e
            return
        e_sb = c.esbp.tile([P, FD], c.f32r, name="e_sb")
        nc.scalar.activation(e_sb[:], st[:], c.AF.Exp, scale=SCALE)
        e_tiles[jc8] = e_sb

    def issue_den(jc8):
        nc.tensor.matmul(den_ps[:], c.ones[:], e_tiles[jc8][:],
                         start=(jc8 == 0), stop=(jc8 == JC8 - 1))

    def issue_pv(jc8):
        for dc in range(DC):
            nc.tensor.matmul(
                pv_ps[dc][:], vh[:, jc8, dc * P:(dc + 1) * P], e_tiles[jc8][:],
                start=(jc8 == 0), stop=(jc8 == JC8 - 1))

    issue_s(0)
    for jc8 in range(1, JC8):
        issue_s(jc8)
        issue_den(jc8 - 1)
        issue_pv(jc8 - 1)
    # denominator completes before the last PV group so the reciprocal
    # overlaps pv(last) on the DVE.
    issue_den(JC8 - 1)
    issue_pv(JC8 - 1)

    if c.pe_only:
        outn = c.d_outn
    else:
        recip = c.recipp.tile([P, FD], c.f32, name="recip")
        nc.vector.reciprocal(recip[:], den_ps[:])
        outn = c.outnp.tile([P, DC, FD], c.f32r, name="outn")
        for dc in range(DC):
            nc.vector.tensor_mul(outn[:, dc, :], pv_ps[dc][:], recip[:])

    # output projection for this (h, ic)
    for ec in range(EC):
        po = c.ps_d.tile([P, FD], c.f32, name="ps_d")
        for dc in range(DC):
            nc.tensor.matmul(
                po[:], wo[:, dc, ec * P:(ec + 1) * P], outn[:, dc, :],
                start=(dc == 0), stop=(dc == DC - 1))
        if c.pe_only:
            continue
        if h == 0:
            nc.vector.tensor_copy(repT[:, ec, i_sl], po[:])
        else:
            nc.vector.tensor_add(repT[:, ec, i_sl], repT[:, ec, i_sl], po[:])


_CACHED_NC = None


def _get_nc():
    global _CACHED_NC
    if _CACHED_NC is None:
        _CACHED_NC = build_core_program()
    return _CACHED_NC


def _prep_in_maps(q, k, v, Wq, bq, Wk, bk, Wv, bv, Wo, bo):
    """Host-side layout prep + sharding. Returns per-core input maps."""
    f32 = np.float32
    qT = np.ascontiguousarray(
        q.reshape(NCORES, BLOC, N, D).transpose(0, 1, 3, 2)).astype(f32, copy=False)
    kT = np.ascontiguousarray(
        k.reshape(NCORES, BLOC, N, D).transpose(0, 1, 3, 2)).astype(f32, copy=False)
    vT = np.ascontiguousarray(
        v.reshape(NCORES, BLOC, N, D).transpose(0, 1, 3, 2)).astype(f32, copy=False)

    WqT = np.ascontiguousarray(Wq.transpose(0, 2, 1)).astype(f32, copy=False)
    WkT = np.ascontiguousarray(Wk.transpose(0, 2, 1)).astype(f32, copy=False)
    WvT = np.ascontiguousarray(Wv.transpose(0, 2, 1)).astype(f32, copy=False)
    # Wo[eo, dd*H + h] -> WoT[h, dd, eo]
    WoT = np.ascontiguousarray(
        Wo.reshape(D, D, H).transpose(2, 1, 0)).astype(f32, copy=False)
    # bq_dev[p, h*EC + ec] = bq[h, ec*128 + p]
    bq_dev = np.ascontiguousarray(
        bq.reshape(H, D // P, P).transpose(2, 0, 1).reshape(P, -1)).astype(f32)
    # bo_eff = bo + sum_h bv[h] @ WoT[h]  (bv folded through output projection)
    bo_eff = bo.astype(f32) + np.einsum(
        "hd,hde->e", bv.astype(np.float64), WoT.astype(np.float64)).astype(f32)
    bo_dev = np.ascontiguousarray(bo_eff.reshape(D // P, P).T).astype(f32)
    ones = np.ones((P, P), f32)

    shared = dict(WqT=WqT, WkT=WkT, WvT=WvT, WoT=WoT,
                  bq_d=bq_dev, bo_d=bo_dev, ones_d=ones)
    return [dict(qT=qT[c], kT=kT[c], vT=vT[c], **shared) for c in range(NCORES)]


def kernel(**inputs):
    nc = _get_nc()
    in_maps = _prep_in_maps(
        inputs["q"], inputs["k"], inputs["v"],
        inputs["Wq"], inputs["bq"], inputs["Wk"], inputs["bk"],
        inputs["Wv"], inputs["bv"], inputs["Wo"], inputs["bo"])
    res = run_bass_kernel_spmd(nc, in_maps, list(range(NCORES)))
    out = np.stack([res.results[c]["outT"] for c in range(NCORES)])  # [8,4,D,N]
    return np.ascontiguousarray(
        out.transpose(0, 1, 3, 2).reshape(B, N, D)).astype(np.float32)


# revision 16
# speedup vs baseline: 1.0004x; 1.0004x over previous
"""Multi-head attention kernel for Trainium2 (Bass/Tile), 8-core data parallel.

Problem: B=32, N=1024, D=512, H=8 (per-head dim = D = 512).
  kh = k @ Wk[h].T + bk ; vh = v @ Wv[h].T + bv ; qh = q @ Wq[h].T + bq
  S = qh @ kh.T / sqrt(D); P = softmax(S); out_h = P @ vh
  rep = concat_interleaved(out_h) @ Wo.T + bo

Sharding: batch data-parallel, 4 batches per core. All math per (b, h) is
done in "transposed" (feature-on-partition) orientation so no on-chip
transposes are needed:
  qhT[e,i]  = matmul(lhsT=WqT, rhs=qT)        (+ bq during PSUM eviction)
  khT[e,j]  = matmul(lhsT=WkT, rhs=kT)        (bk dropped: softmax-invariant)
  vh[j,d]   = matmul(lhsT=vT,  rhs=WvT)       (bv folded into bo on host)
  ST[j,i]   = matmul(lhsT=khT, rhs=qhT)
  E[j,i]    = exp(ST/sqrt(D))                  (no max-subtract: scores ~N(0,1))
  denom     = onesT @ E   (all-ones lhsT -> every row = column sum)
  outT[d,i] = matmul(lhsT=vh, rhs=E) * (1/denom)  (rescale on eviction)
  repT[eo,i]+= matmul(lhsT=WoT_h, rhs=outT)   (accumulate heads in SBUF)
  out = repT + bo_eff                          (host folds bv through Wo)

Matmul operands use float32r (full PE rate at free-dim>=256, ~1.5e-4 rel err).
"""
import math
from contextlib import ExitStack

import numpy as np

import concourse.bacc as bacc
import concourse.mybir as mybir
import concourse.tile as tile
from concourse.bass_utils import run_bass_kernel_spmd

dt = mybir.dt
P = 128

B, N, D, H = 32, 1024, 512, 8
NCORES = 8
BLOC = B // NCORES

FD = 512           # matmul free-dim / PSUM bank width (f32)
SCALE = 1.0 / math.sqrt(D)


class _Ctx:
    pass


def build_core_program(bloc=BLOC, n=N, d=D, h_cnt=H, reps=1, pe_only=False, no_dma=False, ps_s_bufs=3, ps_pv_bufs=4):
    """Bass program for one core: bloc batches, full heads."""
    c = _Ctx()
    c.DC = d // P        # d-partition chunks (4)
    c.EC = d // P        # output-feature chunks (4)
    c.IC = n // FD       # query free-dim chunks (2)
    c.JC8 = n // P       # key partition chunks (8)
    c.n, c.d, c.h_cnt = n, d, h_cnt
    c.pe_only = pe_only
    c.no_dma = no_dma

    nc = bacc.Bacc("TRN2", target_bir_lowering=False, debug=False)
    c.nc = nc

    f32, f32r = dt.float32, dt.float32r
    c.f32, c.f32r = f32, f32r
    c.qT = nc.dram_tensor("qT", [bloc, d, n], f32r, kind="ExternalInput")
    c.kT = nc.dram_tensor("kT", [bloc, d, n], f32r, kind="ExternalInput")
    c.vT = nc.dram_tensor("vT", [bloc, d, n], f32r, kind="ExternalInput")
    c.WqT = nc.dram_tensor("WqT", [h_cnt, d, d], f32r, kind="ExternalInput")
    c.WkT = nc.dram_tensor("WkT", [h_cnt, d, d], f32r, kind="ExternalInput")
    c.WvT = nc.dram_tensor("WvT", [h_cnt, d, d], f32r, kind="ExternalInput")
    c.WoT = nc.dram_tensor("WoT", [h_cnt, d, d], f32r, kind="ExternalInput")
    c.bq_d = nc.dram_tensor("bq_d", [P, h_cnt * c.EC], f32, kind="ExternalInput")
    c.bo_d = nc.dram_tensor("bo_d", [P, c.EC], f32, kind="ExternalInput")
    c.ones_d = nc.dram_tensor("ones_d", [P, P], f32r, kind="ExternalInput")
    c.outT = nc.dram_tensor("outT", [bloc, d, n], f32, kind="ExternalOutput")

    c.AF = mybir.ActivationFunctionType

    with tile.TileContext(nc) as tc, ExitStack() as es:
        ep = es.enter_context
        c.const = ep(tc.tile_pool(name="const", bufs=1))
        c.acts = ep(tc.tile_pool(name="acts", bufs=1))
        c.wqp = ep(tc.tile_pool(name="wq", bufs=2))
        c.wkp = ep(tc.tile_pool(name="wk", bufs=2))
        c.wvp = ep(tc.tile_pool(name="wv", bufs=2))
        c.wop = ep(tc.tile_pool(name="wo", bufs=2))
        c.projp = ep(tc.tile_pool(name="proj", bufs=1))
        c.esbp = ep(tc.tile_pool(name="esb", bufs=4))
        c.outnp = ep(tc.tile_pool(name="outn", bufs=2))
        c.recipp = ep(tc.tile_pool(name="recip", bufs=1))
        c.repp = ep(tc.tile_pool(name="rep", bufs=1))
        c.ps_s = ep(tc.tile_pool(name="ps_s", bufs=ps_s_bufs, space="PSUM"))
        c.ps_pv = ep(tc.tile_pool(name="ps_pv", bufs=ps_pv_bufs, space="PSUM"))
        c.ps_d = ep(tc.tile_pool(name="ps_d", bufs=1, space="PSUM"))

        c.ones = c.const.tile([P, P], f32r, name="ones")
        nc.sync.dma_start(c.ones[:], c.ones_d[:])
        c.bq_sb = c.const.tile([P, h_cnt * c.EC], f32, name="bq_sb")
        nc.sync.dma_start(c.bq_sb[:], c.bq_d[:])
        c.bo_sb = c.const.tile([P, c.EC], f32, name="bo_sb")
        nc.sync.dma_start(c.bo_sb[:], c.bo_d[:])
        if pe_only:
            c.d_qhT = c.const.tile([P, c.EC, n], f32r, name="d_qhT")
            c.d_khT = c.const.tile([P, c.EC, n], f32r, name="d_khT")
            c.d_vh = c.const.tile([P, c.JC8, FD], f32r, name="d_vh")
            c.d_e = c.const.tile([P, FD], f32r, name="d_e")
            c.d_outn = c.const.tile([P, c.DC, FD], f32r, name="d_outn")
            if no_dma:
                c.d_w = c.const.tile([P, c.DC, d], f32r, name="d_w")
                nc.sync.dma_start(c.d_w[:], c.WqT[0].rearrange("(c p) e -> p c e", p=P))
            nc.sync.dma_start(c.d_qhT[:], c.qT[0].rearrange("(c p) n -> p c n", p=P))
            nc.sync.dma_start(c.d_khT[:], c.kT[0].rearrange("(c p) n -> p c n", p=P))
            for j in range(c.JC8):
                nc.sync.dma_start(c.d_vh[:, j, :], c.vT[0, 0:P, 0:FD])
            nc.sync.dma_start(c.d_e[:], c.qT[0, 0:P, 0:FD])
            for dcx in range(c.DC):
                nc.sync.dma_start(c.d_outn[:, dcx, :], c.qT[0, 0:P, 0:FD])

        for rep in range(reps):
            for b in range(bloc):
                _emit_batch(c, b)

    nc.compile()
    return nc


def _emit_batch(c, b):
    nc = c.nc
    qt = c.acts.tile([P, c.DC, c.n], c.f32r, name="qt")
    kt = c.acts.tile([P, c.DC, c.n], c.f32r, name="kt")
    vt = c.acts.tile([P, c.DC, c.n], c.f32r, name="vt")
    # h=0 weights are issued first: the HWDGE queue is serial, and the
    # first projection group needs wq before anything else. Activation
    # chunks follow so proj(q, ec=0) can start after wq + one 512KB chunk.
    w0 = _issue_weight_dmas(c, 0)
    for dcx in range(c.DC):
        nc.sync.dma_start(qt[:, dcx, :], c.qT[b, dcx * P:(dcx + 1) * P, :])
        nc.sync.dma_start(kt[:, dcx, :], c.kT[b, dcx * P:(dcx + 1) * P, :])
        nc.sync.dma_start(vt[:, dcx, :], c.vT[b, dcx * P:(dcx + 1) * P, :])

    repT = c.repp.tile([P, c.EC, c.n], c.f32, name="repT")

    for h in range(c.h_cnt):
        _emit_head(c, h, qt, kt, vt, repT, w0 if h == 0 else None)

    for ec in range(c.EC):
        nc.vector.tensor_scalar_add(
            repT[:, ec, :], repT[:, ec, :], c.bo_sb[:, ec:ec + 1])
    nc.sync.dma_start(
        c.outT[b].rearrange("(c p) n -> p c n", p=P), repT[:])


def _issue_weight_dmas(c, h):
    nc = c.nc
    if c.no_dma:
        return (c.d_w,) * 4
    wq = c.wqp.tile([P, c.DC, c.d], c.f32r, name="wq")
    wk = c.wkp.tile([P, c.DC, c.d], c.f32r, name="wk")
    wv = c.wvp.tile([P, c.DC, c.d], c.f32r, name="wv")
    wo = c.wop.tile([P, c.DC, c.d], c.f32r, name="wo")
    nc.sync.dma_start(wq[:], c.WqT[h].rearrange("(c p) e -> p c e", p=P))
    nc.sync.dma_start(wk[:], c.WkT[h].rearrange("(c p) e -> p c e", p=P))
    nc.sync.dma_start(wv[:], c.WvT[h].rearrange("(c p) e -> p c e", p=P))
    nc.sync.dma_start(wo[:], c.WoT[h].rearrange("(c p) e -> p c e", p=P))
    return wq, wk, wv, wo


def _emit_head(c, h, qt, kt, vt, repT, w0=None):
    nc = c.nc
    DC, EC, IC, JC8 = c.DC, c.EC, c.IC, c.JC8

    wq, wk, wv, wo = w0 if w0 is not None else _issue_weight_dmas(c, h)

    # ---- projections ----
    if c.pe_only:
        qhT, khT, vh = c.d_qhT, c.d_khT, c.d_vh
    else:
        qhT = c.projp.tile([P, EC, c.n], c.f32r, name="qhT")
        khT = c.projp.tile([P, EC, c.n], c.f32r, name="khT")
        vh = c.projp.tile([P, JC8, FD], c.f32r, name="vh")

    for ec in range(EC):
        for ic in range(IC):
            pq = c.ps_s.tile([P, FD], c.f32, name="ps_s")
            for dc in range(DC):
                nc.tensor.matmul(
                    pq[:], wq[:, dc, ec * P:(ec + 1) * P],
                    qt[:, dc, ic * FD:(ic + 1) * FD],
                    start=(dc == 0), stop=(dc == DC - 1))
            if not c.pe_only:
                nc.scalar.activation(
                    qhT[:, ec, ic * FD:(ic + 1) * FD], pq[:], c.AF.Identity,
                    bias=c.bq_sb[:, h * EC + ec:h * EC + ec + 1])
    for ec in range(EC):
        for jc in range(IC):
            pk = c.ps_s.tile([P, FD], c.f32, name="ps_s")
            for dc in range(DC):
                nc.tensor.matmul(
                    pk[:], wk[:, dc, ec * P:(ec + 1) * P],
                    kt[:, dc, jc * FD:(jc + 1) * FD],
                    start=(dc == 0), stop=(dc == DC - 1))
            if not c.pe_only:
                nc.vector.tensor_copy(khT[:, ec, jc * FD:(jc + 1) * FD], pk[:])
    for jc8 in range(JC8):
        pv = c.ps_s.tile([P, FD], c.f32, name="ps_s")
        for dc in range(DC):
            nc.tensor.matmul(
                pv[:], vt[:, dc, jc8 * P:(jc8 + 1) * P], wv[:, dc, :],
                start=(dc == 0), stop=(dc == DC - 1))
        if not c.pe_only:
            nc.vector.tensor_copy(vh[:, jc8, :], pv[:])

    # ---- attention + output projection, per query chunk ----
    if c.pe_only:
        qhT, khT, vh = c.d_qhT, c.d_khT, c.d_vh
    for ic in range(IC):
        _emit_attention_chunk(c, h, ic, qhT, khT, vh, wo, repT)


def _emit_attention_chunk(c, h, ic, qhT, khT, vh, wo, repT):
    nc = c.nc
    DC, EC, JC8 = c.DC, c.EC, c.JC8
    i_sl = slice(ic * FD, (ic + 1) * FD)

    pv_ps = [c.ps_pv.tile([P, FD], c.f32, name="ps_pv") for _ in range(DC)]
    den_ps = c.ps_d.tile([P, FD], c.f32, name="ps_d")
    e_tiles = [None] * JC8

    # software-pipelined: S(j+1) issues on PE while exp(j) runs on ACT,
    # then denom/PV(j) consume.
    def issue_s(jc8):
        st = c.ps_s.tile([P, FD], c.f32, name="ps_s")
        for ec in range(EC):
            nc.tensor.matmul(
                st[:], khT[:, ec, jc8 * P:(jc8 + 1) * P], qhT[:, ec, i_sl],
                start=(ec == 0), stop=(ec == EC - 1))
        if c.pe_only:
            e_tiles[jc8] = c.d_e
            return
        e_sb = c.esbp.tile([P, FD], c.f32r, name="e_sb")
        nc.scalar.activation(e_sb[:], st[:], c.AF.Exp, scale=SCALE)
        e_tiles[jc8] = e_sb

    def issue_den(jc8):
        nc.tensor.matmul(den_ps[:], c.ones[:], e_tiles[jc8][:],
                         start=(jc8 == 0), stop=(jc8 == JC8 - 1))

    def issue_pv(jc8):
        for dc in range(DC):
            nc.tensor.matmul(
                pv_ps[dc][:], vh[:, jc8, dc * P:(dc + 1) * P], e_tiles[jc8][:],
                start=(jc8 == 0), stop=(jc8 == JC8 - 1))

    issue_s(0)
    for jc8 in range(1, JC8):
        issue_s(jc8)
        issue_den(jc8 - 1)
        issue_pv(jc8 - 1)
    # denominator completes before the last PV group so the reciprocal
    # overlaps pv(last) on the DVE.
    issue_den(JC8 - 1)
    issue_pv(JC8 - 1)

    if c.pe_only:
        outn = c.d_outn
    else:
        recip = c.recipp.tile([P, FD], c.f32, name="recip")
        nc.vector.reciprocal(recip[:], den_ps[:])
        outn = c.outnp.tile([P, DC, FD], c.f32r, name="outn")
        for dc in range(DC):
            nc.vector.tensor_mul(outn[:, dc, :], pv_ps[dc][:], recip[:])

    # output projection for this (h, ic)
    for ec in range(EC):
        po = c.ps_d.tile([P, FD], c.f32, name="ps_d")
        for dc in range(DC):
            nc.tensor.matmul(
                po[:], wo[:, dc, ec * P:(ec + 1) * P], outn[:, dc, :],
                start=(dc == 0), stop=(dc == DC - 1))
        if c.pe_only:
            continue
        if h == 0:
            nc.vector.tensor_copy(repT[:, ec, i_sl], po[:])
        else:
            nc.vector.tensor_add(repT[:, ec, i_sl], repT[:, ec, i_sl], po[:])


_CACHED_NC = None


def _get_nc():
    global _CACHED_NC
    if _CACHED_NC is None:
        _CACHED_NC = build_core_program()
    return _CACHED_NC


def _prep_in_maps(q, k, v, Wq, bq, Wk, bk, Wv, bv, Wo, bo):
    """Host-side layout prep + sharding. Returns per-core input maps."""
    f32 = np.float32
    qT = np.ascontiguousarray(
        q.reshape(NCORES, BLOC, N, D).transpose(0, 1, 3, 2)).astype(f32, copy=False)
    kT = np.ascontiguousarray(
        k.reshape(NCORES, BLOC, N, D).transpose(0, 1, 3, 2)).astype(f32, copy=False)
    vT = np.ascontiguousarray(
        v.reshape(NCORES, BLOC, N, D).transpose(0, 1, 3, 2)).astype(f32, copy=False)

    WqT = np.ascontiguousarray(Wq.transpose(0, 2, 1)).astype(f32, copy=False)
    WkT = np.ascontiguousarray(Wk.transpose(0, 2, 1)).astype(f32, copy=False)
    WvT = np.ascontiguousarray(Wv.transpose(0, 2, 1)).astype(f32, copy=False)
    # Wo[eo, dd*H + h] -> WoT[h, dd, eo]
    WoT = np.ascontiguousarray(
        Wo.reshape(D, D, H).transpose(2, 1, 0)).astype(f32, copy=False)
    # bq_dev[p, h*EC + ec] = bq[h, ec*128 + p]
    bq_dev = np.ascontiguousarray(
        bq.reshape(H, D // P, P).transpose(2, 0, 1).reshape(P, -1)).astype(f32)
    # bo_eff = bo + sum_h bv[h] @ WoT[h]  (bv folded through output projection)
    bo_eff = bo.astype(f32) + np.einsum(
        "hd,hde->e", bv.astype(np.float64), WoT.astype(np.float64)).astype(f32)
    bo_dev = np.ascontiguousarray(bo_eff.reshape(D // P, P).T).astype(f32)
    ones = np.ones((P, P), f32)

    shared = dict(WqT=WqT, WkT=WkT, WvT=WvT, WoT=WoT,
                  bq_d=bq_dev, bo_d=bo_dev, ones_d=ones)
    return [dict(qT=qT[c], kT=kT[c], vT=vT[c], **shared) for c in range(NCORES)]


def kernel(**inputs):
    nc = _get_nc()
    in_maps = _prep_in_maps(
        inputs["q"], inputs["k"], inputs["v"],
        inputs["Wq"], inputs["bq"], inputs["Wk"], inputs["bk"],
        inputs["Wv"], inputs["bv"], inputs["Wo"], inputs["bo"])
    res = run_bass_kernel_spmd(nc, in_maps, list(range(NCORES)))
    out = np.stack([res.results[c]["outT"] for c in range(NCORES)])  # [8,4,D,N]
    return np.ascontiguousarray(
        out.transpose(0, 1, 3, 2).reshape(B, N, D)).astype(np.float32)
